# revision 19
# baseline (speedup 1.0000x reference)
"""GAT-style message passing (edge softmax + weighted aggregation) on 8 NeuronCores.

In-situ gather-GEMM design (v3):
  - Per edge slot, the raw bf16 feature row of the source node is gathered
    directly from HBM with a transposed MoE gather (dma_gather transpose=True),
    landing in matmul-lhsT layout [din%128, din//128, slot].
  - Narrow blocks: <= 32 dsts x 4 tiles (512 slots, 2 per src-half), so the
    one-hot S / S^T matrices cost half the HBM traffic of 64-dst blocks.
  - Split GEMM: hs (128 cols) accumulates into a per-block 1-bank PSUM tile
    that stays resident until the edge-weight multiply; el (8 cols) goes to a
    shared per-sub-batch PSUM tile, and the er one-hot expansion matmul
    accumulates er+c into the SAME el PSUM columns (so x = el + er + c is
    produced by the PE with no vector add).
  - w = max(exp(x), exp(0.2 x)) with both exps reading el PSUM directly
    (leakyrelu+exp fused via exp-of-max; softmax max-subtraction dropped:
    logits are O(1)).  The DVE then writes zsb.hs = hs_psum * w in one pass
    (merged PSUM->SBUF copy + edge-weight multiply), and w into zsb's el
    columns (the segsum denominator rides along as 8 extra columns).
  - er per destination is a tiny per-core dense GEMM over gathered local dst
    feature rows (one 512-idx transposed gather per pair of gather-groups),
    with the constant c = c_el + c_er folded in on copy to SBUF.
  - Segment-sum as matmul with a host-built fp8 one-hot S per tile; output
    written per group as a contiguous padded [nb*32, 128] table.

Edges are sorted by dst; dst nodes split into 8 contiguous per-core ranges
with ~equal edge counts; consecutive dsts greedily packed into blocks of
<= 32 dsts and <= 2*128 edge slots per src-half (src < 25000 goes to the A
half so gather indices stay non-negative int16).
"""

import sys

for _p in ("/opt/trn_rl_repo",):
    if _p not in sys.path:
        sys.path.insert(0, _p)

import os

import numpy as np
import ml_dtypes

DBG_NBG = int(os.environ.get("K_NBG_LIMIT", "0"))  # truncate groups if >0
DBG_CORES = int(os.environ.get("K_CORES", "0"))  # run on fewer cores if >0
K_SCRATCH = int(os.environ.get("K_SCRATCH", "16384"))  # swdge ring bytes
K_SUBS = tuple(
    int(x) for x in os.environ.get("K_SUBS", "768,768,512").split(",")
)  # gather chunk sizes per half-group
K_MF = int(os.environ.get("K_MF", "1"))  # merged-msb blocks per sub-batch

import concourse.bass as bass
import concourse.bacc as bacc
import concourse.mybir as mybir
import concourse.tile as tile
from concourse.bass_utils import run_bass_kernel_spmd

BF16 = ml_dtypes.bfloat16
FP8 = ml_dtypes.float8_e4m3
P = 128


class Cfg:
    def __init__(self, n_nodes, d_in, kh, dh, n_cores, sw, tpb, bg, sub, neg_slope=0.2):
        assert d_in % P == 0
        self.n_nodes = n_nodes
        self.d_in = d_in
        self.kh = kh
        self.dh = dh
        self.c = kh * dh  # 128
        self.n_cores = n_cores
        self.sw = sw  # dsts per block
        self.tpb = tpb  # tiles (128 slots) per block
        self.bg = bg  # blocks per gather-group
        self.sub = sub  # blocks per compute sub-batch
        self.neg_slope = neg_slope
        self.kc = d_in // P
        self.zgc = self.c + kh  # 136: hs + el
        self.split = n_nodes // 2
        self.tpa = tpb // 2  # A-half tiles per block
        self.rng_pad = 6400  # uniform local-dst table size (>= max core range)
        assert tpb % 2 == 0 and bg % sub == 0 and bg % 2 == 0
        assert self.split <= 32768 and (n_nodes - self.split) <= 32768
        assert self.c == P


FULL_CFG = Cfg(
    n_nodes=50000, d_in=256, kh=8, dh=16, n_cores=8, sw=32, tpb=4, bg=8, sub=4
)


# ----------------------------------------------------------------------------
# Host-side preprocessing
# ----------------------------------------------------------------------------

def _head_minor_perm(cfg):
    c = np.arange(cfg.c)
    return (c % cfg.kh) * cfg.dh + (c // cfg.kh)


def build_weights(cfg, W_src, b_src, W_dst, b_dst, attn):
    kh, dh, din = cfg.kh, cfg.dh, cfg.d_in
    a_l = attn[:, :dh]
    a_r = attn[:, dh:]
    perm = _head_minor_perm(cfg)

    W_el = np.einsum("kd,kdi->ki", a_l, W_src.reshape(kh, dh, din))  # (kh, din)
    W_er = np.einsum("kd,kdi->ki", a_r, W_dst.reshape(kh, dh, din))
    c_el = np.einsum("kd,kd->k", a_l, b_src.reshape(kh, dh))
    c_er = np.einsum("kd,kd->k", a_r, b_dst.reshape(kh, dh))

    Wz = np.concatenate([W_src[perm].T, W_el.T], axis=1)  # (din, zgc)
    Wz = np.ascontiguousarray(
        Wz.reshape(cfg.kc, P, cfg.zgc).transpose(1, 0, 2)
    ).astype(BF16)  # (P, kc, zgc)
    Wer = np.ascontiguousarray(
        W_er.T.reshape(cfg.kc, P, kh).transpose(1, 0, 2)
    ).astype(BF16)  # (P, kc, kh)

    c_vec = (c_el + c_er).astype(np.float32)
    c_rep = np.tile(c_vec[None, :], (P, 1)).astype(BF16)  # (P, kh)
    b_rep = np.tile(b_src[None, :], (cfg.sw, 1)).astype(np.float32)  # (sw, c)
    return Wz, Wer, c_rep, b_rep


def build_schedule(cfg, src_idx, dst_idx):
    """Sort edges by dst, split dsts across cores, pack blocks, build per-core
    index / one-hot / er-offset arrays (uniform shapes across cores)."""
    E = src_idx.shape[0]
    n = cfg.n_nodes
    order = np.argsort(dst_idx, kind="stable")
    ssrc = src_idx[order].astype(np.int64)
    counts = np.bincount(dst_idx, minlength=n).astype(np.int64)
    starts = np.zeros(n + 1, dtype=np.int64)
    np.cumsum(counts, out=starts[1:])
    acnt = np.bincount(dst_idx[src_idx < cfg.split], minlength=n).astype(np.int64)

    bounds = [0]
    for ci in range(1, cfg.n_cores):
        target = E * ci // cfg.n_cores
        d = int(np.searchsorted(starts, target))
        d = max(bounds[-1], min(d, n))
        bounds.append(d)
    bounds.append(n)

    per_core_blocks = []  # list of list of (d0, ndst, edge_start)
    cap = cfg.tpa * P
    for ci in range(cfg.n_cores):
        d0, d1 = bounds[ci], bounds[ci + 1]
        assert d1 - d0 < cfg.rng_pad
        blocks = []
        d = d0
        while d < d1:
            bstart = d
            ua = ub = 0
            while (
                d < d1
                and (d - bstart) < cfg.sw
                and ua + acnt[d] <= cap
                and ub + (counts[d] - acnt[d]) <= cap
            ):
                ua += int(acnt[d])
                ub += int(counts[d] - acnt[d])
                d += 1
            assert d > bstart, f"dst {d} degree {counts[d]} exceeds block caps"
            blocks.append((bstart, d - bstart, int(starts[bstart])))
        per_core_blocks.append(blocks)

    nb = max(len(b) for b in per_core_blocks)
    nbg = -(-nb // cfg.bg)
    if nbg % 2:
        nbg += 1  # er gathers cover pairs of groups
    nb = nbg * cfg.bg

    nia = cfg.bg * cfg.tpa * P  # gather idxs per half-group
    trash = cfg.rng_pad - 1
    idxa = np.zeros((cfg.n_cores, nbg, nia), dtype=np.int16)
    idxb = np.zeros((cfg.n_cores, nbg, nia), dtype=np.int16)
    s_arr = np.zeros((cfg.n_cores, nb, cfg.tpb, P, cfg.sw), dtype=np.float32)
    st2 = np.zeros((cfg.n_cores, nbg, cfg.sw, cfg.bg * cfg.tpb * P), dtype=np.float32)
    # er dst-local indices: per group, bg*sw rows
    gdl = np.full((cfg.n_cores, nbg, cfg.bg * cfg.sw), trash, dtype=np.int16)

    for ci in range(cfg.n_cores):
        d0c = bounds[ci]
        for bi, (bstart, ndst, estart) in enumerate(per_core_blocks[ci]):
            gi, bj = bi // cfg.bg, bi % cfg.bg
            nedges = int(starts[bstart + ndst] - starts[bstart])
            if nedges > 0:
                esrc = ssrc[estart : estart + nedges]
                dloc = np.repeat(
                    np.arange(ndst), counts[bstart : bstart + ndst]
                ).astype(np.int64)
                isa = esrc < cfg.split
                for half, mask in ((0, isa), (1, ~isa)):
                    hsrc = esrc[mask]
                    hloc = dloc[mask]
                    sl = np.arange(len(hsrc))
                    t = sl // P  # tile within half (0..tpa-1)
                    p = sl % P
                    s_arr[ci, bi, half * cfg.tpa + t, p, hloc] = 1.0
                    st2[
                        ci, gi, hloc,
                        (bj * cfg.tpb + half * cfg.tpa + t) * P + p,
                    ] = 1.0
                    fp = (bj * cfg.tpa + t) * P + p
                    loc = hsrc - (0 if half == 0 else cfg.split)
                    (idxa if half == 0 else idxb)[ci, gi, fp] = loc
            gdl[
                ci, gi, bj * cfg.sw : bj * cfg.sw + ndst
            ] = np.arange(bstart - d0c, bstart - d0c + ndst)

    def wrap16(a):
        # (n_cores, nbg, nia) -> (n_cores, nbg, 128, nia // 16) int16
        w = a.reshape(cfg.n_cores, nbg, nia // 16, 16).transpose(0, 1, 3, 2)
        return np.ascontiguousarray(np.tile(w, (1, 1, 8, 1)))

    idxa = wrap16(idxa)
    idxb = wrap16(idxb)
    # dst-row gather idxs for PAIRS of groups (2*bg*sw = 512 idxs per call):
    # (n_cores, nbg//2, 128, 2*bg*sw//16)
    npp = cfg.bg * cfg.sw
    assert nbg % 2 == 0
    g = gdl.reshape(cfg.n_cores * (nbg // 2), 2 * npp // 16, 16).transpose(0, 2, 1)
    gdl = np.ascontiguousarray(
        np.tile(g, (1, 8, 1)).reshape(cfg.n_cores, nbg // 2, P, 2 * npp // 16)
    )

    # S to SBUF layout: (n_cores, nbg, P, bg*tpb*sw), fp8
    s_arr = (
        s_arr.reshape(cfg.n_cores, nbg, cfg.bg, cfg.tpb, P, cfg.sw)
        .transpose(0, 1, 4, 2, 3, 5)
        .reshape(cfg.n_cores, nbg, P, cfg.bg * cfg.tpb * cfg.sw)
    )
    s_arr = np.ascontiguousarray(s_arr).astype(FP8)
    st2 = st2.astype(FP8)
    meta = dict(bounds=bounds, nb=nb, nbg=nbg, blocks=per_core_blocks)
    return idxa, idxb, s_arr, st2, gdl, meta


# ----------------------------------------------------------------------------
# Device program
# ----------------------------------------------------------------------------

def build_program(cfg, nb, nbg):
    kh, c, kc, zgc = cfg.kh, cfg.c, cfg.kc, cfg.zgc
    sw, tpb, tpa, bg, sub = cfg.sw, cfg.tpb, cfg.tpa, cfg.bg, cfg.sub
    split, rng_pad = cfg.split, cfg.rng_pad
    gpt = bg * tpb  # tiles per group (32)
    spt = sub * tpb  # tiles per sub-batch (16)
    nsub = bg // sub  # sub-batches per group (2)
    nia = bg * tpa * P  # idxs per half-group (2048)
    npp = bg * sw  # er rows per group (256)
    bf = mybir.dt.bfloat16
    f32 = mybir.dt.float32
    fp8 = mybir.dt.float8e4
    i16 = mybir.dt.int16

    ncore = DBG_CORES or cfg.n_cores
    nc = bacc.Bacc(
        "TRN2",
        target_bir_lowering=False,
        debug=False,
        num_devices=ncore,
        dynamic_dma_scratch_size=K_SCRATCH,
    )

    fsA_d = nc.dram_tensor("fsA", [split, cfg.d_in], bf, kind="ExternalInput")
    fsB_d = nc.dram_tensor("fsB", [cfg.n_nodes - split, cfg.d_in], bf, kind="ExternalInput")
    wz_d = nc.dram_tensor("wz", [P, kc, zgc], bf, kind="ExternalInput")
    wer_d = nc.dram_tensor("wer", [P, kc, kh], bf, kind="ExternalInput")
    crep_d = nc.dram_tensor("crep", [P, kh], bf, kind="ExternalInput")
    brep_d = nc.dram_tensor("brep", [sw, c], f32, kind="ExternalInput")
    fdloc_d = nc.dram_tensor("fdloc", [rng_pad, cfg.d_in], bf, kind="ExternalInput")
    idxa_d = nc.dram_tensor("idxa", [nbg, P, nia // 16], i16, kind="ExternalInput")
    idxb_d = nc.dram_tensor("idxb", [nbg, P, nia // 16], i16, kind="ExternalInput")
    s_d = nc.dram_tensor("s", [nbg, P, bg * tpb * sw], fp8, kind="ExternalInput")
    st2_d = nc.dram_tensor("st2", [nbg, sw, bg * tpb * P], fp8, kind="ExternalInput")
    gdl_d = nc.dram_tensor(
        "gdl", [nbg // 2, P, 2 * npp // 16], i16, kind="ExternalInput"
    )

    out_d = nc.dram_tensor("out", [nb * sw, c], f32, kind="ExternalOutput")

    with tile.TileContext(nc) as tc:
        with tc.tile_pool(name="consts", bufs=1) as cpool:
            wz_sb = cpool.tile([P, kc, zgc], bf, name="wz_sb")
            nc.sync.dma_start(out=wz_sb[:], in_=wz_d[:, :, :])
            wer_sb = cpool.tile([P, kc, kh], bf, name="wer_sb")
            nc.sync.dma_start(out=wer_sb[:], in_=wer_d[:, :, :])
            crep_sb = cpool.tile([P, kh], bf, name="crep_sb")
            nc.sync.dma_start(out=crep_sb[:], in_=crep_d[:, :])
            brep_sb = cpool.tile([sw, c], f32, name="brep_sb")
            nc.sync.dma_start(out=brep_sb[:], in_=brep_d[:, :])

            # ---------------- Edge processing ----------------------
            with (
                tc.tile_pool(name="edge", bufs=2) as epool,
                tc.tile_pool(name="inp", bufs=4) as ipool,
                tc.tile_pool(name="hs0", bufs=1, space="PSUM") as hpool0,
                tc.tile_pool(name="hs1", bufs=1, space="PSUM") as hpool1,
                tc.tile_pool(name="hs2", bufs=1, space="PSUM") as hpool2,
                tc.tile_pool(name="hs3", bufs=1, space="PSUM") as hpool3,
                tc.tile_pool(name="elps", bufs=2, space="PSUM") as elpool,
                tc.tile_pool(name="pbps", bufs=1, space="PSUM") as pbpool,
                tc.tile_pool(name="erbps", bufs=1, space="PSUM") as erbpool,
            ):
                hpools = (hpool0, hpool1, hpool2, hpool3)
                for g in range(DBG_NBG or nbg):
                    ia = ipool.tile([P, nia // 16], i16, name="ia", tag="ia")
                    nc.sync.dma_start(out=ia[:], in_=idxa_d[g])
                    ib = ipool.tile([P, nia // 16], i16, name="ib", tag="ib")
                    nc.sync.dma_start(out=ib[:], in_=idxb_d[g])
                    ssb = ipool.tile([P, gpt * sw], fp8, name="ssb", tag="ssb")
                    nc.sync.dma_start(out=ssb[:], in_=s_d[g])
                    st2sb = ipool.tile(
                        [sw, bg * tpb * P], fp8, name="st2sb", tag="st2sb"
                    )
                    nc.sync.dma_start(out=st2sb[:], in_=st2_d[g])

                    # transposed MoE gathers: [din%128, din//128, slot]
                    subs = K_SUBS
                    offs = tuple(sum(K_SUBS[:i]) for i in range(len(K_SUBS)))
                    gA = epool.tile([P, kc * nia], bf, name="gA", tag="gA")
                    gB = epool.tile([P, kc * nia], bf, name="gB", tag="gB")
                    for gt, src_ap, ixt in ((gA, fsA_d, ia), (gB, fsB_d, ib)):
                        for ou, su in zip(offs, subs):
                            nc.gpsimd.dma_gather(
                                out_ap=bass.AP(
                                    gt.tensor,
                                    gt.offset + ou * kc,
                                    [[kc * nia, P], [su, kc], [1, su]],
                                ),
                                in_ap=src_ap[:, :],
                                idxs_ap=ixt[:, ou // 16 : (ou + su) // 16],
                                num_idxs=su,
                                num_idxs_reg=su,
                                elem_size=cfg.d_in,
                                transpose=True,
                            )

                    # er rows for the group pair: one 512-idx transposed
                    # gather of dst feature rows from the per-core local
                    # table, then a tiny GEMM against W_er per block.
                    if g % 2 == 0:
                        gdli = epool.tile(
                            [P, 2 * npp // 16], i16, name="gdli", tag="gdli"
                        )
                        nc.sync.dma_start(out=gdli[:], in_=gdl_d[g // 2])
                        gd = epool.tile([P, kc, 2 * npp], bf, name="gd", tag="gd")
                        nc.gpsimd.dma_gather(
                            out_ap=gd[:, :, :],
                            in_ap=fdloc_d[:, :],
                            idxs_ap=gdli[:, :],
                            num_idxs=2 * npp,
                            num_idxs_reg=2 * npp,
                            elem_size=cfg.d_in,
                            transpose=True,
                        )
                        erb_ps = erbpool.tile(
                            [sw, 2 * bg, kh], f32, name="erb_ps", tag="erb_ps"
                        )
                        for b in range(2 * bg):
                            for k in range(kc):
                                nc.tensor.matmul(
                                    erb_ps[:, b, :],
                                    lhsT=gd[:, k, b * sw : (b + 1) * sw],
                                    rhs=wer_sb[:, k, :],
                                    start=(k == 0),
                                    stop=(k == kc - 1),
                                )
                        # fold in the constant c = c_el + c_er while moving to
                        # SBUF (so the erg expansion matmul emits er + c)
                        erbs = epool.tile([sw, 2 * bg, kh], bf, name="erbs", tag="erbs")
                        nc.vector.tensor_tensor(
                            out=erbs[:, :, :],
                            in0=erb_ps[:, :, :],
                            in1=bass.AP(
                                crep_sb.tensor, crep_sb.offset,
                                [[kh, sw], [0, 2 * bg], [1, kh]],
                            ),
                            op=mybir.AluOpType.add,
                        )
                    go = (g % 2) * bg

                    stg = epool.tile([sw, bg // 2, 2, zgc], bf, name="stg", tag="stg")
                    for q in range(nsub):
                        # hs GEMM per block into resident 1-bank PSUM tiles;
                        # el GEMM + er one-hot expansion accumulate into a
                        # shared el PSUM tile: x = el + er + c.
                        elps = elpool.tile([P, spt, kh], f32, name="elps", tag="elps")
                        hsps = []
                        for j in range(sub):
                            bj = q * sub + j
                            hsp = hpools[j].tile(
                                [P, tpb, c], f32, name=f"hs{j}", tag=f"hs{j}"
                            )
                            hsps.append(hsp)
                            for half, gt in ((0, gA), (1, gB)):
                                for t in range(tpa):
                                    col = (bj * tpa + t) * P
                                    ui = next(
                                        i for i in range(len(subs))
                                        if offs[i] <= col < offs[i] + subs[i]
                                    )
                                    ou, su = offs[ui], subs[ui]
                                    lhs = bass.AP(
                                        gt.tensor,
                                        gt.offset + ou * kc + (col - ou),
                                        [[kc * nia, P], [su, kc], [1, P]],
                                    )
                                    tt = half * tpa + t
                                    for k in range(kc):
                                        nc.tensor.matmul(
                                            hsp[:, tt, :],
                                            lhsT=lhs[:, k, :],
                                            rhs=wz_sb[:, k, 0:c],
                                            start=(k == 0),
                                            stop=(k == kc - 1),
                                        )
                                    for k in range(kc):
                                        nc.tensor.matmul(
                                            elps[:, j * tpb + tt, :],
                                            lhsT=lhs[:, k, :],
                                            rhs=wz_sb[:, k, c:zgc],
                                            start=(k == 0),
                                            stop=False,
                                        )
                                    # er expansion accumulates er + c on top
                                    nc.tensor.matmul(
                                        elps[:, j * tpb + tt, :],
                                        lhsT=st2sb[
                                            :, (bj * tpb + tt) * P
                                            : (bj * tpb + tt + 1) * P
                                        ],
                                        rhs=erbs[:, go + bj, :],
                                        start=False,
                                        stop=True,
                                    )

                        # w = max(exp(x), exp(0.2x)); exps read el PSUM
                        e1 = epool.tile([P, spt, kh], bf, name="e1", tag="e1")
                        nc.scalar.activation(
                            e1[:, :, :], elps[:, :, :],
                            mybir.ActivationFunctionType.Exp,
                        )
                        e2 = epool.tile([P, spt, kh], bf, name="e2", tag="e2")
                        nc.scalar.activation(
                            e2[:, :, :], elps[:, :, :],
                            mybir.ActivationFunctionType.Exp,
                            scale=float(cfg.neg_slope),
                        )
                        zsb = epool.tile(
                            [P, spt, zgc], bf, name=f"zsb{q}", tag=f"zsb{q}"
                        )
                        w_ap = bass.AP(
                            zsb.tensor, zsb.offset + c,
                            [[spt * zgc, P], [zgc, spt], [1, kh]],
                        )
                        nc.vector.tensor_tensor(
                            out=w_ap, in0=e1[:, :, :], in1=e2[:, :, :],
                            op=mybir.AluOpType.max,
                        )
                        # hybrid PSUM drain: first K_MF blocks get the
                        # merged DVE move+multiply from PSUM; the rest are
                        # plain Act copies followed by one in-place bf16
                        # 2x-rate DVE multiply over their zsb range.
                        for j in range(K_MF, sub):
                            nc.scalar.copy(
                                bass.AP(
                                    zsb.tensor, zsb.offset + j * tpb * zgc,
                                    [[spt * zgc, P], [zgc, tpb], [1, c]],
                                ),
                                hsps[j][:, :, :],
                            )
                        for j in range(K_MF):
                            hs_out = bass.AP(
                                zsb.tensor, zsb.offset + j * tpb * zgc,
                                [[spt * zgc, P], [zgc, tpb], [kh, cfg.dh], [1, kh]],
                            )
                            hs_in = bass.AP(
                                hsps[j].tensor, hsps[j].offset,
                                [[tpb * c, P], [c, tpb], [kh, cfg.dh], [1, kh]],
                            )
                            wj_ap = bass.AP(
                                zsb.tensor, zsb.offset + j * tpb * zgc + c,
                                [[spt * zgc, P], [zgc, tpb], [0, cfg.dh], [1, kh]],
                            )
                            nc.vector.tensor_tensor(
                                out=hs_out, in0=hs_in, in1=wj_ap,
                                op=mybir.AluOpType.mult,
                            )
                        if K_MF < sub:
                            rng = bass.AP(
                                zsb.tensor, zsb.offset + K_MF * tpb * zgc,
                                [[spt * zgc, P], [zgc, (sub - K_MF) * tpb],
                                 [kh, cfg.dh], [1, kh]],
                            )
                            wr_ap = bass.AP(
                                zsb.tensor, zsb.offset + K_MF * tpb * zgc + c,
                                [[spt * zgc, P], [zgc, (sub - K_MF) * tpb],
                                 [0, cfg.dh], [1, kh]],
                            )
                            nc.vector.tensor_tensor(
                                out=rng, in0=rng, in1=wr_ap,
                                op=mybir.AluOpType.mult,
                            )

                        # segment-sum matmuls + flush to stg (2 blocks per
                        # PSUM tile, halving the Act drain op count)
                        for jp in range(sub // 2):
                            pb = pbpool.tile([sw, 2, zgc], f32, name="pb", tag="pb")
                            for j2 in range(2):
                                j = jp * 2 + j2
                                bj = q * sub + j
                                for t in range(tpb):
                                    nc.tensor.matmul(
                                        pb[:, j2, :],
                                        lhsT=ssb[
                                            :, (bj * tpb + t) * sw
                                            : (bj * tpb + t + 1) * sw
                                        ],
                                        rhs=zsb[:, j * tpb + t, :],
                                        start=(t == 0),
                                        stop=(t == tpb - 1),
                                    )
                            nc.scalar.copy(
                                stg[:, q * (sub // 2) + jp, :, :], pb[:, :, :]
                            )

                    # normalize + bias + flush (per group)
                    lp = nc.allow_low_precision(reason="bf16 softmax denominators")
                    lp.__enter__()
                    st_t, st_off = stg.tensor, stg.offset
                    den = bass.AP(st_t, st_off + c, [[bg * zgc, sw], [zgc, bg], [1, kh]])
                    nc.vector.tensor_scalar_max(den, den, 1e-20)
                    rcp = epool.tile([sw, bg * kh], bf, name="rcp", tag="rcp")
                    nc.vector.reciprocal(rcp[:], den)

                    outp = epool.tile([sw, bg, c], f32, name="outp", tag="outp")
                    o_t, o_off = outp.tensor, outp.offset
                    num_ap = bass.AP(
                        st_t, st_off, [[bg * zgc, sw], [zgc, bg], [kh, cfg.dh], [1, kh]]
                    )
                    out_ap = bass.AP(
                        o_t, o_off, [[bg * c, sw], [c, bg], [1, cfg.dh], [cfg.dh, kh]]
                    )
                    rcp_ap = bass.AP(
                        rcp.tensor, rcp.offset,
                        [[bg * kh, sw], [kh, bg], [0, cfg.dh], [1, kh]],
                    )
                    nc.vector.tensor_tensor(
                        out=out_ap, in0=num_ap, in1=rcp_ap, op=mybir.AluOpType.mult
                    )
                    b_ap = bass.AP(
                        brep_sb.tensor, brep_sb.offset, [[c, sw], [0, bg], [1, c]]
                    )
                    ofl = bass.AP(o_t, o_off, [[bg * c, sw], [1, bg * c]])
                    nc.vector.tensor_tensor(
                        out=ofl, in0=ofl, in1=b_ap, op=mybir.AluOpType.add
                    )
                    nc.sync.dma_start(
                        out=bass.AP(
                            out_d.ap().tensor,
                            g * bg * sw * c,
                            [[c, sw], [sw * c, bg], [1, c]],
                        ),
                        in_=outp[:, :, :],
                    )
                    lp.__exit__(None, None, None)

    nc.compile()
    return nc


# ----------------------------------------------------------------------------
# Entry point
# ----------------------------------------------------------------------------

def _run(cfg, inputs, trace=False):
    feat_src = np.asarray(inputs["feat_src"], dtype=np.float32)
    feat_dst = np.asarray(inputs["feat_dst"], dtype=np.float32)
    W_src = np.asarray(inputs["W_src"], dtype=np.float32)
    b_src = np.asarray(inputs["b_src"], dtype=np.float32)
    W_dst = np.asarray(inputs["W_dst"], dtype=np.float32)
    b_dst = np.asarray(inputs["b_dst"], dtype=np.float32)
    attn = np.asarray(inputs["attn"], dtype=np.float32)
    src_idx = np.asarray(inputs["src_idx"]).astype(np.int64)
    dst_idx = np.asarray(inputs["dst_idx"]).astype(np.int64)

    Wz, Wer, c_rep, b_rep = build_weights(cfg, W_src, b_src, W_dst, b_dst, attn)
    idxa, idxb, s_arr, st2, gdl, meta = build_schedule(cfg, src_idx, dst_idx)
    nb, nbg, bounds = meta["nb"], meta["nbg"], meta["bounds"]

    feat_bf = feat_src.astype(BF16)
    fsA = np.ascontiguousarray(feat_bf[: cfg.split])
    fsB = np.ascontiguousarray(feat_bf[cfg.split :])

    fdloc = np.zeros((cfg.n_cores, cfg.rng_pad, cfg.d_in), dtype=BF16)
    for ci in range(cfg.n_cores):
        d0, d1 = bounds[ci], bounds[ci + 1]
        fdloc[ci, : d1 - d0] = feat_dst[d0:d1].astype(BF16)

    nc = build_program(cfg, nb, nbg)

    in_maps = []
    for ci in range(cfg.n_cores):
        in_maps.append(
            {
                "fsA": fsA,
                "fsB": fsB,
                "wz": Wz,
                "wer": Wer,
                "crep": c_rep,
                "brep": b_rep,
                "fdloc": fdloc[ci],
                "idxa": idxa[ci],
                "idxb": idxb[ci],
                "s": s_arr[ci],
                "st2": st2[ci],
                "gdl": gdl[ci],
            }
        )

    ncore = DBG_CORES or cfg.n_cores
    res = run_bass_kernel_spmd(
        nc, in_maps[:ncore], core_ids=list(range(ncore)), trace=trace
    )

    out = np.zeros((cfg.n_nodes, cfg.c), dtype=np.float32)
    for ci in range(DBG_CORES or cfg.n_cores):
        tab = res.results[ci]["out"]  # [nb*sw, c]
        for bi, (bstart, ndst, _) in enumerate(meta["blocks"][ci]):
            out[bstart : bstart + ndst] = tab[bi * cfg.sw : bi * cfg.sw + ndst]
    deg = np.bincount(dst_idx, minlength=cfg.n_nodes)
    out[deg == 0] = 0.0
    return out, res


def kernel(**inputs) -> np.ndarray:
    out, _ = _run(FULL_CFG, inputs, trace=False)
    return out
